# revision 5
# baseline (speedup 1.0000x reference)
"""Trainium2 Bass kernel for bidirectional cross-attention (nn_CrossAttention).

Reference computation (per batch b, N=1024 tokens, D=768 dims):
    sim1  = image1 @ image2^T            [N, N]
    out2  = l2norm(softmax(sim1) @ image2) + 2*image2
    sim2  = image2 @ image1^T
    out1  = l2norm(softmax(sim2) @ image1) + 2*image1

Two algebraic simplifications:
  1. l2norm(softmax(S) @ V) == l2norm(exp(S - rowmax) @ V): the softmax
     denominator is a positive per-row scalar cancelled by the L2 norm,
     so the kernel never computes the softmax sum.
  2. sim2 == sim1^T: the kernel computes S = image1 @ image2^T ONCE per
     batch and derives direction 2's scores by PE-transposing the fp16
     SBUF copy of S (8 transposes/tile) instead of a second 49k-cycle
     matmul.

Precision plan (rel-err gate is 2e-2; expected ~1e-3):
  - inputs cast fp32->fp16 during DMA; mm1 in fp16 (1 cycle/row on PE,
    same speed as bf16, ~8x less S error)
  - dir1 softmax: rowmax + exp read S from PSUM fp32 directly
  - dir2 softmax: from the fp16 transpose of S (+-0.03 abs error in the
    exponent -> ~3% on P entries; output is residual-dominated)
  - P matrices stored bf16, transposed on PE, cast to fp8e4 on PSUM
    evacuation
  - both mm2 (P^T.T @ V) run in fp8e4 DoubleRow perf mode: K=256 packed
    per instruction = 2x PE throughput; V in fp8e4 (|V| <= ~5.5 << 448)

Sharding: pure data parallel, B=16 batches -> 2 per core across 8 cores.

PSUM budget (8 banks x 2KB): acc pool 3 x [128,1024]f32 slots shared by
mm1's S tiles and mm2's O tiles (6 banks) + tp16 1 x [128,8,128]f16
(input transposes + S^T groups) + tpP 1 x [128,8,128]bf16 (P transposes).

DIR-phase software pipeline (i = 0..8) per batch:
  dir1(i): P1T transposes + fp8 mm2 -> out2[i]
  ST(i):   S^T transpose group for dir2 row-block i (rowmax DVE, exp ACT)
  dir2(i-1): P2T transposes + fp8 mm2 -> out1[i-1]  (the ST(i-1) ->
             rowmax -> exp chain hides under dir1(i)+ST(i) PE work)
Batch b+1's loads fire at i==0 (ring depth 2 means they only conflict
with batch b-1's readers, which are long done); its input transposes are
injected 2 per iteration. Engine split: ACT = S16 copy, exps, P1T evac,
Square; DVE = rowmaxes, P2T evac, input-transpose evac, residual
doubling, epilogue stt; GPSIMD = load DMA triggers + fp8 natural copies.
"""

import os
import sys

import numpy as np

for _p in ("/opt/trn_rl_repo", "/root/.axon_site/_ro/trn_rl_repo"):
    if os.path.isdir(_p) and _p not in sys.path:
        sys.path.append(_p)

B, N, D = 16, 1024, 768
NCORES = 8
BPC = B // NCORES  # batches per core
P = 128
NT = N // P  # 8 token chunks
DT = D // P  # 6 feature chunks

_PROGRAM_CACHE = {}


def build_program():
    """Build the per-core Bass program (SPMD: identical on all cores)."""
    import concourse.mybir as mybir
    import concourse.tile as tile
    from concourse import bacc
    from concourse.masks import make_identity

    f32 = mybir.dt.float32
    f16 = mybir.dt.float16
    bf16 = mybir.dt.bfloat16
    f8 = mybir.dt.float8e4
    AF = mybir.ActivationFunctionType
    ALU = mybir.AluOpType
    AX = mybir.AxisListType
    DR = mybir.MatmulPerfMode.DoubleRow

    nc = bacc.Bacc(None)
    img_dram = {
        1: nc.declare_dram_parameter("image1", [BPC, N, D], f32, isOutput=False),
        2: nc.declare_dram_parameter("image2", [BPC, N, D], f32, isOutput=False),
    }
    out_dram = {
        1: nc.declare_dram_parameter("out1", [BPC, N, D], f32, isOutput=True),
        2: nc.declare_dram_parameter("out2", [BPC, N, D], f32, isOutput=True),
    }

    with tile.TileContext(nc) as tc:
        with (
            tc.tile_pool(name="const", bufs=1) as const_pool,
            tc.tile_pool(name="nat", bufs=2) as nat_pool,
            tc.tile_pool(name="imgT", bufs=2) as imgT_pool,
            tc.tile_pool(name="s16", bufs=2) as s16_pool,
            tc.tile_pool(name="p1", bufs=NT) as p1_pool,
            tc.tile_pool(name="pw", bufs=2) as pw_pool,
            tc.tile_pool(name="work", bufs=3) as work,
            tc.tile_pool(name="outs", bufs=4) as outs,
            tc.tile_pool(name="stats", bufs=8) as stats,
            tc.tile_pool(name="acc", bufs=3, space="PSUM") as acc_pool,
            tc.tile_pool(name="tp16", bufs=1, space="PSUM") as tp16_pool,
            tc.tile_pool(name="tpP", bufs=1, space="PSUM") as tpP_pool,
        ):
            ident16 = const_pool.tile([P, P], f16, tag="id16")
            make_identity(nc, ident16[:])
            identb = const_pool.tile([P, P], bf16, tag="idb")
            make_identity(nc, identb[:])

            nat16 = {}  # (b, im) -> list of 8 fp16 natural chunks [P, D]
            nat8 = {}   # (b, im) -> [P, NT, D] fp8 natural copy (mm2 rhs)
            imgT = {}   # (b, im) -> [P, DT, N] fp16 transposed (mm1 operands)
            s16 = {}    # (b, qi) -> [P, N] fp16 copy of S row-block

            def prep_loads(b):
                """Trigger all cast-DMA loads (fp32->fp16), then fp8 natural
                copies (gpsimd waits each landed chunk in order).
                image2 chunks first: mm1's rhs needs imgT2 complete."""
                for im in (2, 1):
                    chunks = []
                    nat8[(b, im)] = nat_pool.tile(
                        [P, NT, D], f8, tag=f"nat8_{im}", name=f"nat8_{im}"
                    )
                    for kc in range(NT):
                        nb = nat_pool.tile(
                            [P, D], f16, tag=f"nat16_{im}_{kc}", name="nb"
                        )
                        nc.gpsimd.dma_start(
                            nb[:], img_dram[im][b, kc * P : (kc + 1) * P, :]
                        )
                        chunks.append(nb)
                    nat16[(b, im)] = chunks
                for im in (2, 1):
                    for kc in range(NT):
                        nc.gpsimd.tensor_copy(
                            nat8[(b, im)][:, kc, :], nat16[(b, im)][kc][:]
                        )

            def prep_groups(b):
                """Return 16 closures, each PE-transposing one (im, kc) chunk
                into column kc of imgT[im] (6 blocks -> [P, dc, kc*P:...])."""
                tbs = {}
                for im in (2, 1):
                    tbs[im] = imgT_pool.tile(
                        [P, DT, N], f16, tag=f"imgT{im}", name=f"imgT{im}"
                    )
                    imgT[(b, im)] = tbs[im]

                def make(im, kc):
                    def g():
                        nb = nat16[(b, im)][kc]
                        tp = tp16_pool.tile([P, N], f16, tag="tp16")
                        for dc in range(DT):
                            nc.tensor.transpose(
                                tp[:, dc * P : (dc + 1) * P],
                                nb[:, dc * P : (dc + 1) * P],
                                ident16[:],
                            )
                        for dc in range(DT):
                            nc.vector.tensor_copy(
                                tbs[im][:, dc, kc * P : (kc + 1) * P],
                                tp[:, dc * P : (dc + 1) * P],
                            )
                    return g

                return [make(im, kc) for im in (2, 1) for kc in range(NT)]

            def mm1(b, qi):
                """S[qi,:] = img1^T.T @ img2^T (fp16), then S16 copy (ACT),
                rowmax (DVE), P1 = exp(S - rowmax) (ACT, fp32 PSUM read)."""
                S = acc_pool.tile([P, N], f32, tag="acc")
                qT = imgT[(b, 1)]
                kT = imgT[(b, 2)]
                for d in range(DT):
                    lhsT = qT[:, d, qi * P : (qi + 1) * P]
                    nc.tensor.matmul(
                        S[:, :512], lhsT, kT[:, d, :512],
                        start=(d == 0), stop=(d == DT - 1),
                    )
                    nc.tensor.matmul(
                        S[:, 512:], lhsT, kT[:, d, 512:],
                        start=(d == 0), stop=(d == DT - 1),
                    )
                sb = s16_pool.tile([P, N], f16, tag=f"s16_{qi}", name="sb")
                s16[(b, qi)] = sb
                nc.scalar.activation(sb[:], S[:], AF.Copy)
                negmax = stats.tile([P, 1], f32, tag="negmax1")
                nc.vector.tensor_reduce(
                    negmax, S[:], axis=AX.X, op=ALU.max, negate=True
                )
                Pw = p1_pool.tile([P, N], bf16, tag="P1")
                nc.scalar.activation(Pw, S[:], AF.Exp, bias=negmax, scale=1.0)
                return Pw

            def mm2(PTs, v8, resid_nat, out_ap):
                """O = P^T.T @ V in fp8 DoubleRow; l2norm + 2*resid epilogue."""
                Ot = acc_pool.tile([P, N], f32, tag="acc")
                for c in range(3):
                    cs = slice(c * 256, (c + 1) * 256)
                    for g in range(4):
                        nc.tensor.matmul(
                            Ot[:, cs],
                            PTs[:, 2 * g : 2 * g + 2, :],
                            v8[:, 2 * g : 2 * g + 2, cs],
                            start=(g == 0), stop=(g == 3),
                            perf_mode=DR,
                        )
                # epilogue: out = O * rsqrt(sum(O^2)) + 2*resid
                sq = work.tile([P, D], f32, tag="sq")
                ss = stats.tile([P, 1], f32, tag="ss")
                nc.scalar.activation(sq, Ot[:, :D], AF.Square, accum_out=ss)
                s2 = stats.tile([P, 1], f32, tag="s2")
                nc.scalar.activation(s2, ss, AF.Sqrt)
                inv = stats.tile([P, 1], f32, tag="inv")
                nc.vector.reciprocal(inv, s2)
                resid2 = work.tile([P, D], f16, tag="resid2")
                nc.vector.tensor_scalar_mul(resid2[:], resid_nat[:], 2.0)
                T3 = outs.tile([P, D], f32, tag="T3")
                nc.vector.scalar_tensor_tensor(
                    out=T3, in0=Ot[:, :D], scalar=inv, in1=resid2[:],
                    op0=ALU.mult, op1=ALU.add,
                )
                nc.sync.dma_start(out_ap, T3[:])

            def dir1_iter(b, qi, Pw):
                """P1T transposes (bf16), fp8 evac on ACT, mm2 -> out2[qi]."""
                tp = tpP_pool.tile([P, N], bf16, tag="tpP")
                for kc in range(NT):
                    nc.tensor.transpose(
                        tp[:, kc * P : (kc + 1) * P],
                        Pw[:, kc * P : (kc + 1) * P], identb[:]
                    )
                PTs = pw_pool.tile([P, NT, P], f8, tag="P1Ts")
                nc.scalar.activation(PTs[:], tp[:], AF.Copy)
                mm2(
                    PTs, nat8[(b, 2)], nat16[(b, 2)][qi],
                    out_dram[2][b, qi * P : (qi + 1) * P, :],
                )

            def st_group(b, mi):
                """Transpose S16 column-block mi -> ST psum [P, NT, P] fp16,
                then rowmax (DVE) + exp (ACT) -> P2 bf16."""
                tp = tp16_pool.tile([P, N], f16, tag="tp16")
                for qi in range(NT):
                    nc.tensor.transpose(
                        tp[:, qi * P : (qi + 1) * P],
                        s16[(b, qi)][:, mi * P : (mi + 1) * P],
                        ident16[:],
                    )
                negmax = stats.tile([P, 1], f32, tag="negmax2")
                nc.vector.tensor_reduce(
                    negmax, tp[:], axis=AX.X, op=ALU.max, negate=True
                )
                P2 = pw_pool.tile([P, N], bf16, tag="P2")
                nc.scalar.activation(P2, tp[:], AF.Exp, bias=negmax, scale=1.0)
                return P2

            def dir2_iter(b, mi, P2):
                """P2T transposes (bf16), fp8 evac on DVE, mm2 -> out1[mi]."""
                tp = tpP_pool.tile([P, NT, P], bf16, tag="tpP")
                for kc in range(NT):
                    nc.tensor.transpose(
                        tp[:, kc, :], P2[:, kc * P : (kc + 1) * P], identb[:]
                    )
                PTs = pw_pool.tile([P, NT, P], f8, tag="P2Ts")
                nc.vector.tensor_copy(PTs[:], tp[:])
                mm2(
                    PTs, nat8[(b, 1)], nat16[(b, 1)][mi],
                    out_dram[1][b, mi * P : (mi + 1) * P, :],
                )

            # ---- schedule ----
            prep_loads(0)
            for g in prep_groups(0):
                g()
            for b in range(BPC):
                P1s = {qi: mm1(b, qi) for qi in range(NT)}
                pending_groups = []
                P2_prev = None
                for i in range(NT + 1):
                    if i < NT:
                        dir1_iter(b, i, P1s.pop(i))
                        P2_cur = st_group(b, i)
                    else:
                        P2_cur = None
                    if i == 0 and b + 1 < BPC:
                        prep_loads(b + 1)
                        pending_groups = prep_groups(b + 1)
                    if P2_prev is not None:
                        dir2_iter(b, i - 1, P2_prev)
                    P2_prev = P2_cur
                    if pending_groups and i >= 1:
                        for g in pending_groups[:2]:
                            g()
                        pending_groups = pending_groups[2:]

    return nc


def _get_program():
    if "nc" not in _PROGRAM_CACHE:
        nc = build_program()
        if not nc.is_finalized():
            nc.finalize()
        _PROGRAM_CACHE["nc"] = nc
    return _PROGRAM_CACHE["nc"]


def kernel(image1: np.ndarray, image2: np.ndarray):
    from concourse.bass_utils import run_bass_kernel_spmd

    image1 = np.ascontiguousarray(image1, dtype=np.float32)
    image2 = np.ascontiguousarray(image2, dtype=np.float32)
    assert image1.shape == (B, N, D) and image2.shape == (B, N, D)

    nc = _get_program()
    core_ids = list(range(NCORES))
    in_maps = [
        {
            "image1": image1[c * BPC : (c + 1) * BPC],
            "image2": image2[c * BPC : (c + 1) * BPC],
        }
        for c in core_ids
    ]
    res = run_bass_kernel_spmd(nc, in_maps, core_ids)
    out1 = np.concatenate([res.results[c]["out1"] for c in core_ids], axis=0)
    out2 = np.concatenate([res.results[c]["out2"] for c in core_ids], axis=0)
    return out1, out2
